# revision 35
# baseline (speedup 1.0000x reference)
"""Trainium2 Bass kernel for channel-attention (nn_Attention_77094662963280).

Reference math (per batch b, x_b: [N=16384, C=192], heads: c = hd*6+m, hd<32, m<6):
    qkv = x @ w_qkv^T ; q,k,v split
    score[hd,m,l] = sum_n q[n, hd*6+m] k[n, hd*6+l] * HD^-0.5      (6x6 per (b,hd))
    weight = softmax(score, -1)
    out[n, hd*6+m] = sum_l weight[hd,m,l] v[n, hd*6+l]
    y = out @ w_proj^T

Key algebraic restructure (everything after the Gram matrix is linear):
    G_b   = x_b^T x_b                                   [C,C]   (pass 1)
    scoreF= w_q G_b w_k^T                               [C,C];  block-diag 6x6 blocks are the scores
    Wblk  = softmax over masked rows of scoreF          [C,C]   (0 off-block)
    W2_b  = (w_proj @ Wblk) @ w_v                       [C,C]
    y_b   = x_b @ W2_b^T                                        (pass 2)

So each batch needs exactly two streaming passes over x (read once: pass 1 keeps a
transposed bf16 copy of x resident in SBUF for pass 2) and a tiny per-batch fixup.
Sharding: data-parallel over batch B=16 across 8 cores (2 batches/core), weights
replicated. No collectives.

Optimizations vs the f32/bf16 baseline (176.7us -> ~127us):
  - host pre-casts x to fp16 and pre-tiles x/out partition-major [B_LOC,P,NT,C]
    (3KB contiguous DMA lines; halves the HBM read). fp16 (10 mantissa bits)
    keeps the score path accurate where bf16 fails: Gram entries are ~16k while
    softmax logits come from cancellation.
  - symmetric-Gram: the second Gram matmul streams only the 64-wide diagonal
    block (N=64); the off-diagonal block is rebuilt from G1^T by PE transposes
    in the interlude (the computed G is bitwise symmetric).
  - the whole interlude runs fp16 at uniform K=128/M=128 geometry: chunk-1
    operands live on partitions 64:128 of zero-padded [128,C] tiles (non-
    uniform K/M or partition-offset outputs serialize the PE at ~270ns/op).
  - wq is pre-scaled by TEMP on the host; negate=True on the max-reduce feeds
    the exp bias directly, shortening the softmax chain.
  - x loads ride sync's HWDGE ring (first 4 groups pre-issued before the SWDGE
    weight loads); stores ride sync too; PSUM->SBUF copies split vector/scalar;
    pass-2 PSUM tiles rotate through two pools (6 buffers) and ysb has 4
    buffers so the store receipt latency never WARs the tail.
  - interlude stages are emitted under a cover plan (s4's softmax chain gets
    two compute groups of cover) so their cross-engine chains hide.
"""

import os
import sys

import numpy as np

for _p in ("/opt/trn_rl_repo", "/opt/pypackages"):
    if os.path.isdir(_p) and _p not in sys.path:
        sys.path.append(_p)

import ml_dtypes

import concourse.bass as bass
import concourse.tile as tile
from concourse import bacc, mybir
from concourse.bass_utils import run_bass_kernel_spmd

B, H, W, C = 16, 128, 128, 192
N = H * W                 # 16384 spatial positions
M = 6                     # heads
HD = C // M               # 32
TEMP = float(HD) ** -0.5
NCORES = 8
B_LOC = B // NCORES       # 2 batches per core
P = 128                   # partition tile
NT = N // P               # 128 n-tiles per batch
TD = 8                    # n-tiles per DMA group
FP = mybir.dt.float32
BF = mybir.dt.float16  # on-chip 16-bit dtype (fp16: 10 mantissa bits, 1 cy/row on PE)
MASK_NEG = -1.0e9


def _load_mat_pair(nc, consts, dram_ap, name, cast=True):
    """Load a [192,192] DRAM matrix into two [128,192] SBUF chunks.

    Chunk 0 holds rows 0:128.  Chunk 1 holds rows 128:192 on PARTITIONS
    64:128 with zeros on partitions 0:64 -- the partition-offset convention
    that keeps every interlude matmul at uniform K=128/M=128 (the zero rows
    annihilate whatever garbage sits in the paired operand's low half).
    SWDGE (gpsimd) casting DMAs write the fp16 chunks directly, so nothing
    sits on the sync/scalar HWDGE queues at startup.
    """
    dt = BF if cast else FP
    eng = nc.gpsimd if cast else nc.sync
    t0 = consts.tile([P, C], dt, tag=f"{name}_0")
    eng.dma_start(out=t0[:, :], in_=dram_ap[0:P, :])
    t1 = consts.tile([P, C], dt, tag=f"{name}_1")
    nc.vector.memset(t1[0:64, :], 0.0)
    eng.dma_start(out=t1[64:P, :], in_=dram_ap[P:C, :])
    return [t0, t1]


def build_kernel():
    nc = bacc.Bacc("TRN2", target_bir_lowering=False, debug=False)

    x_d = nc.declare_dram_parameter("x", [B_LOC, P, NT, C], BF, isOutput=False)
    wqT_d = nc.declare_dram_parameter("wqT", [C, C], FP, isOutput=False)
    wkT_d = nc.declare_dram_parameter("wkT", [C, C], FP, isOutput=False)
    wv_d = nc.declare_dram_parameter("wv", [C, C], FP, isOutput=False)
    wprojT_d = nc.declare_dram_parameter("wprojT", [C, C], FP, isOutput=False)
    mask_d = nc.declare_dram_parameter("mask", [C, C], FP, isOutput=False)
    ident_d = nc.declare_dram_parameter("ident", [P, P], FP, isOutput=False)
    out_d = nc.declare_dram_parameter("out", [B_LOC, P, NT, C], BF, isOutput=True)

    x_ap = x_d.ap()
    out_ap = out_d.ap()
    NG = NT // TD  # DMA groups per pass (16)

    with tile.TileContext(nc) as tc:
        with (
            tc.tile_pool(name="consts", bufs=1) as consts,
            tc.tile_pool(name="wstage", bufs=2) as wstage,
            tc.tile_pool(name="xbf", bufs=5) as xbf_pool,
            tc.tile_pool(name="xta", bufs=2) as xta_pool,
            tc.tile_pool(name="xtb", bufs=2) as xtb_pool,
            tc.tile_pool(name="ysb", bufs=4) as ysb_pool,
            tc.tile_pool(name="interm", bufs=2) as interm,
            tc.tile_pool(name="w2t", bufs=4) as w2t_pool,
            tc.tile_pool(name="scal", bufs=8) as scal,
            tc.tile_pool(name="tp_ps", bufs=3, space="PSUM") as tp_pool,
            tc.tile_pool(name="g1_ps", bufs=1, space="PSUM") as g1_pool,
            tc.tile_pool(name="g2_ps", bufs=1, space="PSUM") as g2_pool,
            tc.tile_pool(name="y_ps", bufs=3, space="PSUM") as y_pool,
        ):
            warm_rhs = consts.tile([P, 512], BF, tag="warm_rhs")
            nc.vector.memset(warm_rhs[:, :], 0.0)
            warm_w = consts.tile([P, P], BF, tag="warm_w")
            nc.vector.memset(warm_w[:, :], 0.0)

            def warm_pe(n_mm):
                wps = y_pool.tile([P, 512], FP, tag="y", name="warmps")
                for i in range(n_mm):
                    nc.tensor.matmul(
                        wps[:, :], warm_w[:, :], warm_rhs[:, :],
                        start=(i == 0), stop=(i == n_mm - 1),
                    )

            # ---------------- per-batch state & load pre-issue ------------
            state = {}

            def p1_start(b):
                st = {}
                st["g1"] = g1_pool.tile([P, C], FP, tag="g1", name="gacc1")
                st["g2"] = g2_pool.tile([P, 64], FP, tag="g2", name="gacc2")
                st["xta"] = xta_pool.tile([P, NT, P], BF, tag="xta", name="xta")
                st["xtb"] = xtb_pool.tile([P, NT, P], BF, tag="xtb", name="xtb")
                st["xb_pre"] = {}
                state[b] = st

            def issue_load(b, g, eng=None):
                xb = xbf_pool.tile([P, TD, C], BF, tag="xb")
                (eng or nc.sync).dma_start(
                    out=xb[:, :, :], in_=x_ap[b, :, g * TD : (g + 1) * TD, :]
                )
                state[b]["xb_pre"][g] = xb

            # the first x tiles are the critical path at kernel start: issue
            # their loads before the (interlude-only) weight loads.
            p1_start(0)
            for _g in range(4):
                issue_load(0, _g)
            warm_pe(14)

            # ident gates the very first p1 transposes -- load it first, via a
            # SWDGE casting DMA (no staging, nothing on the sync/scalar rings)
            ident = consts.tile([P, P], BF, tag="ident")
            nc.gpsimd.dma_start(out=ident[:, :], in_=ident_d.ap()[:, :])

            wqT = _load_mat_pair(nc, consts, wqT_d.ap(), "wqT")
            wkT = _load_mat_pair(nc, consts, wkT_d.ap(), "wkT")
            wv = _load_mat_pair(nc, consts, wv_d.ap(), "wv")
            wprojT = _load_mat_pair(nc, consts, wprojT_d.ap(), "wprojT")
            mask = _load_mat_pair(nc, consts, mask_d.ap(), "mask", cast=False)

            def p1_group(b, g, tp_alt=False):
                st = state[b]
                g1_ps, g2_ps, xta, xtb = st["g1"], st["g2"], st["xta"], st["xtb"]
                # host pre-tiled layout: every partition line is TD*C*2 = 6KB
                # contiguous, full HBM line rate.
                if g in st["xb_pre"]:
                    xb = st["xb_pre"].pop(g)
                else:
                    xb = xbf_pool.tile([P, TD, C], BF, tag="xb")
                    nc.sync.dma_start(out=xb[:, :, :], in_=x_ap[b, :, g * TD : (g + 1) * TD, :])
                for j4 in range(TD // 4):
                    # all 8 transposes of the subgroup share one PSUM bank.
                    # NOTE: every PE op keeps uniform K=128/M=128 stationary and
                    # full-partition outputs -- non-uniform geometry (K=64
                    # matmuls, partition-offset transpose outputs) measurably
                    # serializes the PE (~270ns/op instead of back-to-back).
                    # While pass 2 is idle (tp_alt), rotate through the y PSUM
                    # pool as well: 6 buffers kill the transpose-copy WAR stall.
                    if tp_alt and j4 % 2 == 1:
                        tp = y_pool.tile([P, 8, P], BF, tag="y")
                    else:
                        tp = tp_pool.tile([P, 8, P], BF, tag="tp")
                    for k in range(4):
                        j = j4 * 4 + k
                        nc.tensor.transpose(tp[:, k, :], xb[:, j, 0:P], ident[:, :])
                    for k in range(4):
                        j = j4 * 4 + k
                        nc.tensor.transpose(tp[:, 4 + k, :], xb[:, j, 64:C], ident[:, :])
                    for k in range(4):
                        j = j4 * 4 + k
                        t = g * TD + j
                        nc.tensor.matmul(
                            g1_ps[:, :], xb[:, j, 0:P], xb[:, j, :],
                            start=(t == 0), stop=(t == NT - 1),
                        )
                    for k in range(4):
                        j = j4 * 4 + k
                        t = g * TD + j
                        # symmetric-Gram: only the diagonal 64-wide block of the
                        # lower row-chunk is computed here (N=64 stream); rows
                        # 64:128 of g2 are G[128:192, 128:192].
                        nc.tensor.matmul(
                            g2_ps[:, :], xb[:, j, 64:C], xb[:, j, 128:C],
                            start=(t == 0), stop=(t == NT - 1),
                        )
                    t0 = g * TD + j4 * 4
                    # rotate the PSUM->SBUF copies between DVE and ACT
                    # (gpsimd cannot read PSUM)
                    if (g + j4) % 2 == 0:
                        nc.vector.tensor_copy(xta[:, t0 : t0 + 4, :], tp[:, 0:4, :])
                        nc.scalar.copy(xtb[:, t0 : t0 + 4, :], tp[:, 4:8, :])
                    else:
                        nc.scalar.copy(xta[:, t0 : t0 + 4, :], tp[:, 0:4, :])
                        nc.vector.tensor_copy(xtb[:, t0 : t0 + 4, :], tp[:, 4:8, :])

            def interlude_stages(b):
                st = state[b]
                g1_ps, g2_ps = st["g1"], st["g2"]
                ctx = {}

                # Convention: every "chunk-1" operand (rows 128:192 of a [C,C]
                # object) lives on PARTITIONS 64:128 of a full [128, ...] tile.
                # Weight chunk-1 tiles are zero on partitions 0:64, so data
                # tiles may carry garbage there -- the zero weight rows kill
                # those products.  Every matmul is then uniform K=128/M=128
                # (the M=128 "bb" matmuls recompute rows 64:128 redundantly,
                # like the Gram overlap trick).

                def s1():
                    # g_a = fp16(G[0:128, :]); g_b partitions 64:128 hold
                    # G[128:192, :]: cols 0:128 = (g_a[:,128:192])^T (symmetry),
                    # cols 128:192 = cols 64:128 of (g2_sb)^T (symmetry again).
                    # Both PE transposes write at partition offset 64 (slightly
                    # serialized, but only twice per batch).
                    g_a = interm.tile([P, C], BF, tag="g_a")
                    nc.vector.tensor_copy(g_a[:, :], g1_ps[:, :])
                    g2_sb = interm.tile([P, 64], BF, tag="g2_sb")
                    nc.scalar.copy(g2_sb[:, :], g2_ps[:, :])
                    gt_ps = tp_pool.tile([P, 2, P], BF, tag="tp", name="gt")
                    nc.tensor.transpose(gt_ps[64:P, 0, :], g_a[:, P:C], ident[:, :])
                    nc.tensor.transpose(gt_ps[64:P, 1, :], g2_sb[:, :], ident[:, :])
                    g_b = interm.tile([P, C], BF, tag="g_b")
                    nc.vector.memset(g_b[0:64, :], 0.0)
                    nc.scalar.copy(g_b[64:P, 0:P], gt_ps[64:P, 0, :])
                    nc.vector.tensor_copy(g_b[64:P, P:C], gt_ps[64:P, 1, 64:P])
                    ctx["g_a"], ctx["g_b"] = g_a, g_b

                def s2():
                    g_a, g_b = ctx["g_a"], ctx["g_b"]
                    sc1_ps = tp_pool.tile([P, 2 * C], FP, tag="tp")
                    a, bb = sc1_ps[:, 0:C], sc1_ps[:, C : 2 * C]
                    nc.tensor.matmul(a, g_a[:, 0:P], wkT[0][:, :], start=True, stop=False)
                    nc.tensor.matmul(a, g_b[:, 0:P], wkT[1][:, :], start=False, stop=True)
                    nc.tensor.matmul(bb, g_a[:, 64:C], wkT[0][:, :], start=True, stop=False)
                    nc.tensor.matmul(bb, g_b[:, 64:C], wkT[1][:, :], start=False, stop=True)
                    sc1_a = interm.tile([P, C], BF, tag="sc1_a")
                    sc1_b = interm.tile([P, C], BF, tag="sc1_b")
                    nc.vector.memset(sc1_b[0:64, :], 0.0)
                    nc.scalar.copy(sc1_a[:, :], a)
                    nc.scalar.copy(sc1_b[64:P, :], bb[64:P, :])
                    ctx["sc1_a"], ctx["sc1_b"] = sc1_a, sc1_b

                def s3():
                    sc1_a, sc1_b = ctx["sc1_a"], ctx["sc1_b"]
                    sf_ps = tp_pool.tile([P, 2 * C], FP, tag="tp")
                    a, bb = sf_ps[:, 0:C], sf_ps[:, C : 2 * C]
                    nc.tensor.matmul(a, wqT[0][:, 0:P], sc1_a[:, :], start=True, stop=False)
                    nc.tensor.matmul(a, wqT[1][:, 0:P], sc1_b[:, :], start=False, stop=True)
                    nc.tensor.matmul(bb, wqT[0][:, 64:C], sc1_a[:, :], start=True, stop=False)
                    nc.tensor.matmul(bb, wqT[1][:, 64:C], sc1_b[:, :], start=False, stop=True)
                    ctx["sf_a"], ctx["sf_b"] = a, bb

                def s4():
                    # wq is pre-scaled by TEMP on the host, so sm is already
                    # the softmax logit; negate=True on the max-reduce yields
                    # the exp bias directly (two fewer chain ops).
                    wblk = []
                    for ci, (sfp, lo) in enumerate(((ctx["sf_a"], 0), (ctx["sf_b"], 64))):
                        sm = interm.tile([P, C], FP, tag=f"sm_{ci}")
                        nc.vector.tensor_add(sm[lo:P, :], sfp[lo:P, :], mask[ci][lo:P, :])
                        mx = scal.tile([P, 1], FP, tag=f"mx_{ci}")
                        nc.vector.tensor_reduce(mx[lo:P, :], sm[lo:P, :], axis=mybir.AxisListType.X, op=mybir.AluOpType.max, negate=True)
                        wb = interm.tile([P, C], BF, tag=f"wblk_{ci}")
                        if lo:
                            nc.vector.memset(wb[0:lo, :], 0.0)
                        rs = scal.tile([P, 1], FP, tag=f"rs_{ci}")
                        nc.scalar.activation(
                            out=wb[lo:P, :], in_=sm[lo:P, :],
                            func=mybir.ActivationFunctionType.Exp,
                            bias=mx[lo:P, :], scale=1.0, accum_out=rs[lo:P, :],
                        )
                        rr = scal.tile([P, 1], FP, tag=f"rr_{ci}")
                        nc.vector.reciprocal(rr[lo:P, :], rs[lo:P, :])
                        nc.vector.tensor_scalar_mul(wb[lo:P, :], wb[lo:P, :], rr[lo:P, :])
                        wblk.append(wb)
                    ctx["wblk"] = wblk

                def s5():
                    wblk = ctx["wblk"]
                    we_ps = tp_pool.tile([P, 2 * C], FP, tag="tp")
                    a, bb = we_ps[:, 0:C], we_ps[:, C : 2 * C]
                    nc.tensor.matmul(a, wblk[0][:, 0:P], wprojT[0][:, :], start=True, stop=False)
                    nc.tensor.matmul(a, wblk[1][:, 0:P], wprojT[1][:, :], start=False, stop=True)
                    nc.tensor.matmul(bb, wblk[0][:, 64:C], wprojT[0][:, :], start=True, stop=False)
                    nc.tensor.matmul(bb, wblk[1][:, 64:C], wprojT[1][:, :], start=False, stop=True)
                    we_a = interm.tile([P, C], BF, tag="we_a")
                    we_b = interm.tile([P, C], BF, tag="we_b")
                    nc.vector.memset(we_b[0:64, :], 0.0)
                    nc.scalar.copy(we_a[:, :], a)
                    nc.scalar.copy(we_b[64:P, :], bb[64:P, :])
                    ctx["we_a"], ctx["we_b"] = we_a, we_b

                def s6():
                    we_a, we_b = ctx["we_a"], ctx["we_b"]
                    w2_ps = tp_pool.tile([P, 2 * C], FP, tag="tp")
                    a, bb = w2_ps[:, 0:C], w2_ps[:, C : 2 * C]
                    nc.tensor.matmul(a, wv[0][:, 0:P], we_a[:, :], start=True, stop=False)
                    nc.tensor.matmul(a, wv[1][:, 0:P], we_b[:, :], start=False, stop=True)
                    nc.tensor.matmul(bb, wv[0][:, 64:C], we_a[:, :], start=True, stop=False)
                    nc.tensor.matmul(bb, wv[1][:, 64:C], we_b[:, :], start=False, stop=True)
                    w2t_a = w2t_pool.tile([P, C], BF, tag="w2t_a")
                    w2t_b = w2t_pool.tile([P, C], BF, tag="w2t_b")
                    nc.scalar.copy(w2t_a[:, :], a)
                    nc.vector.tensor_copy(w2t_b[64:P, :], bb[64:P, :])
                    nc.vector.memset(w2t_b[0:64, :], 0.0)
                    st["w2t_a"], st["w2t_b"] = w2t_a, w2t_b

                return [s1, s2, s3, s4, s5, s6]

            def p2_group(b, g, free_g=False):
                st = state[b]
                xta, xtb = st["xta"], st["xtb"]
                w2t_a, w2t_b = st["w2t_a"], st["w2t_b"]
                ysb = ysb_pool.tile([P, TD, C], BF, tag="ysb")
                last = b == 1 and g == NG - 1
                for j2 in range(TD // 2):
                    # tail phase (b==1): rotate PSUM tiles through both pools so
                    # 6 buffers are in flight and the PE never waits on reuse.
                    # In phase C (free_g: interlude(b1) has consumed g1/g2) the
                    # dead Gram accumulator banks join the rotation instead --
                    # tp is busy with interlude stage tiles there.
                    if b == 1:
                        pool = y_pool if j2 % 2 == 0 else tp_pool
                        tag = "y" if j2 % 2 == 0 else "tp"
                    elif free_g:
                        pool = (y_pool, g1_pool, y_pool, g2_pool)[j2 % 4]
                        tag = ("y", "g1", "y", "g2")[j2 % 4]
                    else:
                        pool, tag = y_pool, "y"
                    y_ps = pool.tile([P, 2, C], FP, tag=tag)
                    for k in range(2):
                        t = g * TD + j2 * 2 + k
                        nc.tensor.matmul(y_ps[:, k, :], xta[:, t, :], w2t_a[:, :], start=True, stop=False)
                        nc.tensor.matmul(y_ps[:, k, :], xtb[:, t, :], w2t_b[:, :], start=False, stop=True)
                    j0 = j2 * 2
                    if j2 % 2 == 0:
                        nc.scalar.copy(ysb[:, j0 : j0 + 2, :], y_ps[:, :, :])
                    else:
                        nc.vector.tensor_copy(ysb[:, j0 : j0 + 2, :], y_ps[:, :, :])
                    if last:
                        # drain the final group in [P,2,C] slices right behind
                        # each copy so the tail stub is one slice, not a group
                        eng = nc.sync if j2 % 2 == 0 else nc.scalar
                        eng.dma_start(
                            out=out_ap[b, :, g * TD + j0 : g * TD + j0 + 2, :],
                            in_=ysb[:, j0 : j0 + 2, :],
                        )
                if not last:
                    dst = out_ap[b, :, g * TD : (g + 1) * TD, :]
                    nc.sync.dma_start(out=dst, in_=ysb[:, :, :])

            # ---------------- emission schedule (keeps PE gap-free) --------
            # (p1_start(0) and the first two loads were issued before the
            # weight loads above)
            for g in range(NG):
                p1_group(0, g)
            # interlude(b0) stages interleaved with the first pass-1 groups of b1
            p1_start(1)
            st0 = interlude_stages(0)
            # cover plan: PE work emitted before each stage so its cross-engine
            # chain hides; s3 has no chain (PSUM to PSUM via SBUF weights), s4's
            # softmax chain is the longest and gets double cover.
            COVER = [1, 1, 0, 2, 1, 1]
            gi = 0
            for s, cov in zip(st0, COVER):
                for _ in range(cov):
                    p1_group(1, gi)
                    gi += 1
                s()
            # pass2(b0) interleaved with the rest of pass1(b1)
            g2i = 0
            for g in range(gi, NG):
                p1_group(1, g)
                p2_group(0, g2i)
                g2i += 1
            # interlude(b1) interleaved with the remaining pass2(b0) groups
            st1 = interlude_stages(1)
            first_c = True
            for s, cov in zip(st1, COVER):
                for _ in range(cov):
                    if g2i < NG:
                        p2_group(0, g2i, free_g=not first_c)
                        first_c = False
                        g2i += 1
                s()
            while g2i < NG:
                p2_group(0, g2i)
                g2i += 1
            for g in range(NG):
                p2_group(1, g)

    nc.compile()
    return nc


def _host_inputs(x, w_qkv, w_proj):
    w_q = w_qkv[0:C]
    w_k = w_qkv[C : 2 * C]
    w_v = w_qkv[2 * C : 3 * C]
    p = np.arange(C)
    mask = np.where((p[:, None] // M) == (p[None, :] // M), 0.0, MASK_NEG).astype(
        np.float32
    )
    common = {
        "wqT": np.ascontiguousarray(w_q.T) * TEMP,
        "wkT": np.ascontiguousarray(w_k.T),
        "wv": np.ascontiguousarray(w_v),
        "wprojT": np.ascontiguousarray(w_proj.T),
        "mask": mask,
        "ident": np.eye(P, dtype=np.float32),
    }
    # partition-major bf16 tiling: [B, NT, P, C] -> [B, P, NT, C]
    xt = (
        x.reshape(B, NT, P, C)
        .astype(np.float16)
        .transpose(0, 2, 1, 3)
    )
    in_maps = []
    for i in range(NCORES):
        m = dict(common)
        m["x"] = np.ascontiguousarray(xt[i * B_LOC : (i + 1) * B_LOC])
        in_maps.append(m)
    return in_maps


_CACHED_NC = None


def _get_nc():
    global _CACHED_NC
    if _CACHED_NC is None:
        _CACHED_NC = build_kernel()
    return _CACHED_NC


def kernel(x, w_qkv, w_proj, _trace=False, _results_out=None):
    x = np.ascontiguousarray(np.asarray(x, dtype=np.float32))
    w_qkv = np.asarray(w_qkv, dtype=np.float32)
    w_proj = np.asarray(w_proj, dtype=np.float32)
    nc = _get_nc()
    in_maps = _host_inputs(x, w_qkv, w_proj)
    res = run_bass_kernel_spmd(nc, in_maps, core_ids=list(range(NCORES)), trace=_trace)
    if _results_out is not None:
        _results_out.append(res)
    # out: [B_LOC, P, NT, C] partition-major -> [B_LOC, NT, P, C] -> [B, H, W, C]
    outs = [
        np.asarray(res.results[i]["out"]).astype(np.float32).transpose(0, 2, 1, 3)
        for i in range(NCORES)
    ]
    y = np.concatenate(outs, axis=0).reshape(B, H, W, C)
    return y.astype(np.float32)


# revision 36
# speedup vs baseline: 1.0317x; 1.0317x over previous
"""Trainium2 Bass kernel for channel-attention (nn_Attention_77094662963280).

Reference math (per batch b, x_b: [N=16384, C=192], heads: c = hd*6+m, hd<32, m<6):
    qkv = x @ w_qkv^T ; q,k,v split
    score[hd,m,l] = sum_n q[n, hd*6+m] k[n, hd*6+l] * HD^-0.5      (6x6 per (b,hd))
    weight = softmax(score, -1)
    out[n, hd*6+m] = sum_l weight[hd,m,l] v[n, hd*6+l]
    y = out @ w_proj^T

Key algebraic restructure (everything after the Gram matrix is linear):
    G_b   = x_b^T x_b                                   [C,C]   (pass 1)
    scoreF= w_q G_b w_k^T                               [C,C];  block-diag 6x6 blocks are the scores
    Wblk  = softmax over masked rows of scoreF          [C,C]   (0 off-block)
    W2_b  = (w_proj @ Wblk) @ w_v                       [C,C]
    y_b   = x_b @ W2_b^T                                        (pass 2)

So each batch needs exactly two streaming passes over x (read once: pass 1 keeps a
transposed bf16 copy of x resident in SBUF for pass 2) and a tiny per-batch fixup.
Sharding: data-parallel over batch B=16 across 8 cores (2 batches/core), weights
replicated. No collectives.

Optimizations vs the f32/bf16 baseline (176.7us -> ~127us):
  - host pre-casts x to fp16 and pre-tiles x/out partition-major [B_LOC,P,NT,C]
    (3KB contiguous DMA lines; halves the HBM read). fp16 (10 mantissa bits)
    keeps the score path accurate where bf16 fails: Gram entries are ~16k while
    softmax logits come from cancellation.
  - symmetric-Gram: the second Gram matmul streams only the 64-wide diagonal
    block (N=64); the off-diagonal block is rebuilt from G1^T by PE transposes
    in the interlude (the computed G is bitwise symmetric).
  - the whole interlude runs fp16 at uniform K=128/M=128 geometry: chunk-1
    operands live on partitions 64:128 of zero-padded [128,C] tiles (non-
    uniform K/M or partition-offset outputs serialize the PE at ~270ns/op).
  - wq is pre-scaled by TEMP on the host; negate=True on the max-reduce feeds
    the exp bias directly, shortening the softmax chain.
  - x loads ride sync's HWDGE ring (first 4 groups pre-issued before the SWDGE
    weight loads); stores ride sync too; PSUM->SBUF copies split vector/scalar;
    pass-2 PSUM tiles rotate through two pools (6 buffers) and ysb has 4
    buffers so the store receipt latency never WARs the tail.
  - interlude stages are emitted under a cover plan (s4's softmax chain gets
    two compute groups of cover) so their cross-engine chains hide.
"""

import os
import sys

import numpy as np

for _p in ("/opt/trn_rl_repo", "/opt/pypackages"):
    if os.path.isdir(_p) and _p not in sys.path:
        sys.path.append(_p)

import ml_dtypes

import concourse.bass as bass
import concourse.tile as tile
from concourse import bacc, mybir
from concourse.bass_utils import run_bass_kernel_spmd

B, H, W, C = 16, 128, 128, 192
N = H * W                 # 16384 spatial positions
M = 6                     # heads
HD = C // M               # 32
TEMP = float(HD) ** -0.5
NCORES = 8
B_LOC = B // NCORES       # 2 batches per core
P = 128                   # partition tile
NT = N // P               # 128 n-tiles per batch
TD = 8                    # n-tiles per DMA group
FP = mybir.dt.float32
BF = mybir.dt.float16  # on-chip 16-bit dtype (fp16: 10 mantissa bits, 1 cy/row on PE)
MASK_NEG = -1.0e9


def _load_mat_pair(nc, consts, dram_ap, name, cast=True):
    """Load a [192,192] DRAM matrix into two [128,192] SBUF chunks.

    Chunk 0 holds rows 0:128.  Chunk 1 holds rows 128:192 on PARTITIONS
    64:128 with zeros on partitions 0:64 -- the partition-offset convention
    that keeps every interlude matmul at uniform K=128/M=128 (the zero rows
    annihilate whatever garbage sits in the paired operand's low half).
    SWDGE (gpsimd) casting DMAs write the fp16 chunks directly, so nothing
    sits on the sync/scalar HWDGE queues at startup.
    """
    dt = BF if cast else FP
    eng = nc.gpsimd if cast else nc.sync
    t0 = consts.tile([P, C], dt, tag=f"{name}_0")
    eng.dma_start(out=t0[:, :], in_=dram_ap[0:P, :])
    t1 = consts.tile([P, C], dt, tag=f"{name}_1")
    nc.vector.memset(t1[0:64, :], 0.0)
    eng.dma_start(out=t1[64:P, :], in_=dram_ap[P:C, :])
    return [t0, t1]


def build_kernel():
    nc = bacc.Bacc("TRN2", target_bir_lowering=False, debug=False)

    x_d = nc.declare_dram_parameter("x", [B_LOC, P, NT, C], BF, isOutput=False)
    wqT_d = nc.declare_dram_parameter("wqT", [C, C], FP, isOutput=False)
    wkT_d = nc.declare_dram_parameter("wkT", [C, C], FP, isOutput=False)
    wv_d = nc.declare_dram_parameter("wv", [C, C], FP, isOutput=False)
    wprojT_d = nc.declare_dram_parameter("wprojT", [C, C], FP, isOutput=False)
    mask_d = nc.declare_dram_parameter("mask", [C, C], FP, isOutput=False)
    ident_d = nc.declare_dram_parameter("ident", [P, P], FP, isOutput=False)
    out_d = nc.declare_dram_parameter("out", [B_LOC, P, NT, C], BF, isOutput=True)

    x_ap = x_d.ap()
    out_ap = out_d.ap()
    NG = NT // TD  # DMA groups per pass (16)

    with tile.TileContext(nc) as tc:
        with (
            tc.tile_pool(name="consts", bufs=1) as consts,
            tc.tile_pool(name="wstage", bufs=2) as wstage,
            tc.tile_pool(name="xbf", bufs=5) as xbf_pool,
            tc.tile_pool(name="xta", bufs=2) as xta_pool,
            tc.tile_pool(name="xtb", bufs=2) as xtb_pool,
            tc.tile_pool(name="ysb", bufs=4) as ysb_pool,
            tc.tile_pool(name="interm", bufs=2) as interm,
            tc.tile_pool(name="w2t", bufs=4) as w2t_pool,
            tc.tile_pool(name="scal", bufs=8) as scal,
            tc.tile_pool(name="tp_ps", bufs=3, space="PSUM") as tp_pool,
            tc.tile_pool(name="g1_ps", bufs=1, space="PSUM") as g1_pool,
            tc.tile_pool(name="g2_ps", bufs=1, space="PSUM") as g2_pool,
            tc.tile_pool(name="y_ps", bufs=3, space="PSUM") as y_pool,
        ):
            warm_rhs = consts.tile([P, 512], BF, tag="warm_rhs")
            nc.vector.memset(warm_rhs[:, :], 0.0)
            warm_w = consts.tile([P, P], BF, tag="warm_w")
            nc.vector.memset(warm_w[:, :], 0.0)

            def warm_pe(n_mm):
                wps = y_pool.tile([P, 512], FP, tag="y", name="warmps")
                for i in range(n_mm):
                    nc.tensor.matmul(
                        wps[:, :], warm_w[:, :], warm_rhs[:, :],
                        start=(i == 0), stop=(i == n_mm - 1),
                    )

            # ---------------- per-batch state & load pre-issue ------------
            state = {}

            def p1_start(b):
                st = {}
                st["g1"] = g1_pool.tile([P, C], FP, tag="g1", name="gacc1")
                st["g2"] = g2_pool.tile([P, 64], FP, tag="g2", name="gacc2")
                st["xta"] = xta_pool.tile([P, NT, P], BF, tag="xta", name="xta")
                st["xtb"] = xtb_pool.tile([P, NT, P], BF, tag="xtb", name="xtb")
                st["xb_pre"] = {}
                state[b] = st

            def issue_load(b, g, eng=None):
                xb = xbf_pool.tile([P, TD, C], BF, tag="xb")
                (eng or nc.sync).dma_start(
                    out=xb[:, :, :], in_=x_ap[b, :, g * TD : (g + 1) * TD, :]
                )
                state[b]["xb_pre"][g] = xb

            # the first x tiles are the critical path at kernel start: issue
            # their loads before the (interlude-only) weight loads.
            p1_start(0)
            for _g in range(4):
                issue_load(0, _g)
            warm_pe(14)

            # ident gates the very first p1 transposes -- load it first, via a
            # SWDGE casting DMA (no staging, nothing on the sync/scalar rings)
            ident = consts.tile([P, P], BF, tag="ident")
            nc.gpsimd.dma_start(out=ident[:, :], in_=ident_d.ap()[:, :])

            wqT = _load_mat_pair(nc, consts, wqT_d.ap(), "wqT")
            wkT = _load_mat_pair(nc, consts, wkT_d.ap(), "wkT")
            wv = _load_mat_pair(nc, consts, wv_d.ap(), "wv")
            wprojT = _load_mat_pair(nc, consts, wprojT_d.ap(), "wprojT")
            mask = _load_mat_pair(nc, consts, mask_d.ap(), "mask", cast=False)

            def p1_group(b, g, tp_alt=False):
                st = state[b]
                g1_ps, g2_ps, xta, xtb = st["g1"], st["g2"], st["xta"], st["xtb"]
                # host pre-tiled layout: every partition line is TD*C*2 = 6KB
                # contiguous, full HBM line rate.
                if g in st["xb_pre"]:
                    xb = st["xb_pre"].pop(g)
                else:
                    xb = xbf_pool.tile([P, TD, C], BF, tag="xb")
                    nc.sync.dma_start(out=xb[:, :, :], in_=x_ap[b, :, g * TD : (g + 1) * TD, :])
                for j4 in range(TD // 4):
                    # all 8 transposes of the subgroup share one PSUM bank.
                    # NOTE: every PE op keeps uniform K=128/M=128 stationary and
                    # full-partition outputs -- non-uniform geometry (K=64
                    # matmuls, partition-offset transpose outputs) measurably
                    # serializes the PE (~270ns/op instead of back-to-back).
                    # While pass 2 is idle (tp_alt), rotate through the y PSUM
                    # pool as well: 6 buffers kill the transpose-copy WAR stall.
                    if tp_alt and j4 % 2 == 1:
                        tp = y_pool.tile([P, 8, P], BF, tag="y")
                    else:
                        tp = tp_pool.tile([P, 8, P], BF, tag="tp")
                    for k in range(4):
                        j = j4 * 4 + k
                        nc.tensor.transpose(tp[:, k, :], xb[:, j, 0:P], ident[:, :])
                    for k in range(4):
                        j = j4 * 4 + k
                        nc.tensor.transpose(tp[:, 4 + k, :], xb[:, j, 64:C], ident[:, :])
                    for k in range(4):
                        j = j4 * 4 + k
                        t = g * TD + j
                        nc.tensor.matmul(
                            g1_ps[:, :], xb[:, j, 0:P], xb[:, j, :],
                            start=(t == 0), stop=(t == NT - 1),
                        )
                    for k in range(4):
                        j = j4 * 4 + k
                        t = g * TD + j
                        # symmetric-Gram: only the diagonal 64-wide block of the
                        # lower row-chunk is computed here (N=64 stream); rows
                        # 64:128 of g2 are G[128:192, 128:192].
                        nc.tensor.matmul(
                            g2_ps[:, :], xb[:, j, 64:C], xb[:, j, 128:C],
                            start=(t == 0), stop=(t == NT - 1),
                        )
                    t0 = g * TD + j4 * 4
                    # rotate the PSUM->SBUF copies between DVE and ACT
                    # (gpsimd cannot read PSUM)
                    if (g + j4) % 2 == 0:
                        nc.vector.tensor_copy(xta[:, t0 : t0 + 4, :], tp[:, 0:4, :])
                        nc.scalar.copy(xtb[:, t0 : t0 + 4, :], tp[:, 4:8, :])
                    else:
                        nc.scalar.copy(xta[:, t0 : t0 + 4, :], tp[:, 0:4, :])
                        nc.vector.tensor_copy(xtb[:, t0 : t0 + 4, :], tp[:, 4:8, :])

            def interlude_stages(b):
                st = state[b]
                g1_ps, g2_ps = st["g1"], st["g2"]
                ctx = {}

                # Convention: every "chunk-1" operand (rows 128:192 of a [C,C]
                # object) lives on PARTITIONS 64:128 of a full [128, ...] tile.
                # Weight chunk-1 tiles are zero on partitions 0:64, so data
                # tiles may carry garbage there -- the zero weight rows kill
                # those products.  Every matmul is then uniform K=128/M=128
                # (the M=128 "bb" matmuls recompute rows 64:128 redundantly,
                # like the Gram overlap trick).

                def s1():
                    # g_a = fp16(G[0:128, :]); g_b partitions 64:128 hold
                    # G[128:192, :]: cols 0:128 = (g_a[:,128:192])^T (symmetry),
                    # cols 128:192 = cols 64:128 of (g2_sb)^T (symmetry again).
                    # Both PE transposes write at partition offset 64 (slightly
                    # serialized, but only twice per batch).
                    g_a = interm.tile([P, C], BF, tag="g_a")
                    nc.vector.tensor_copy(g_a[:, :], g1_ps[:, :])
                    g2_sb = interm.tile([P, 64], BF, tag="g2_sb")
                    nc.scalar.copy(g2_sb[:, :], g2_ps[:, :])
                    gt_ps = tp_pool.tile([P, 2, P], BF, tag="tp", name="gt")
                    nc.tensor.transpose(gt_ps[64:P, 0, :], g_a[:, P:C], ident[:, :])
                    nc.tensor.transpose(gt_ps[64:P, 1, :], g2_sb[:, :], ident[:, :])
                    g_b = interm.tile([P, C], BF, tag="g_b")
                    nc.vector.memset(g_b[0:64, :], 0.0)
                    nc.scalar.copy(g_b[64:P, 0:P], gt_ps[64:P, 0, :])
                    nc.vector.tensor_copy(g_b[64:P, P:C], gt_ps[64:P, 1, 64:P])
                    ctx["g_a"], ctx["g_b"] = g_a, g_b

                def s2():
                    g_a, g_b = ctx["g_a"], ctx["g_b"]
                    sc1_ps = tp_pool.tile([P, 2 * C], FP, tag="tp")
                    a, bb = sc1_ps[:, 0:C], sc1_ps[:, C : 2 * C]
                    nc.tensor.matmul(a, g_a[:, 0:P], wkT[0][:, :], start=True, stop=False)
                    nc.tensor.matmul(a, g_b[:, 0:P], wkT[1][:, :], start=False, stop=True)
                    nc.tensor.matmul(bb, g_a[:, 64:C], wkT[0][:, :], start=True, stop=False)
                    nc.tensor.matmul(bb, g_b[:, 64:C], wkT[1][:, :], start=False, stop=True)
                    sc1_a = interm.tile([P, C], BF, tag="sc1_a")
                    sc1_b = interm.tile([P, C], BF, tag="sc1_b")
                    nc.vector.memset(sc1_b[0:64, :], 0.0)
                    nc.scalar.copy(sc1_a[:, :], a)
                    nc.scalar.copy(sc1_b[64:P, :], bb[64:P, :])
                    ctx["sc1_a"], ctx["sc1_b"] = sc1_a, sc1_b

                def s3():
                    sc1_a, sc1_b = ctx["sc1_a"], ctx["sc1_b"]
                    sf_ps = tp_pool.tile([P, 2 * C], FP, tag="tp")
                    a, bb = sf_ps[:, 0:C], sf_ps[:, C : 2 * C]
                    nc.tensor.matmul(a, wqT[0][:, 0:P], sc1_a[:, :], start=True, stop=False)
                    nc.tensor.matmul(a, wqT[1][:, 0:P], sc1_b[:, :], start=False, stop=True)
                    nc.tensor.matmul(bb, wqT[0][:, 64:C], sc1_a[:, :], start=True, stop=False)
                    nc.tensor.matmul(bb, wqT[1][:, 64:C], sc1_b[:, :], start=False, stop=True)
                    ctx["sf_a"], ctx["sf_b"] = a, bb

                def s4():
                    # wq is pre-scaled by TEMP on the host, so sm is already
                    # the softmax logit; negate=True on the max-reduce yields
                    # the exp bias directly (two fewer chain ops).
                    wblk = []
                    for ci, (sfp, lo) in enumerate(((ctx["sf_a"], 0), (ctx["sf_b"], 64))):
                        sm = interm.tile([P, C], FP, tag=f"sm_{ci}")
                        nc.vector.tensor_add(sm[lo:P, :], sfp[lo:P, :], mask[ci][lo:P, :])
                        mx = scal.tile([P, 1], FP, tag=f"mx_{ci}")
                        nc.vector.tensor_reduce(mx[lo:P, :], sm[lo:P, :], axis=mybir.AxisListType.X, op=mybir.AluOpType.max, negate=True)
                        wb = interm.tile([P, C], BF, tag=f"wblk_{ci}")
                        if lo:
                            nc.vector.memset(wb[0:lo, :], 0.0)
                        rs = scal.tile([P, 1], FP, tag=f"rs_{ci}")
                        nc.scalar.activation(
                            out=wb[lo:P, :], in_=sm[lo:P, :],
                            func=mybir.ActivationFunctionType.Exp,
                            bias=mx[lo:P, :], scale=1.0, accum_out=rs[lo:P, :],
                        )
                        rr = scal.tile([P, 1], FP, tag=f"rr_{ci}")
                        nc.vector.reciprocal(rr[lo:P, :], rs[lo:P, :])
                        nc.vector.tensor_scalar_mul(wb[lo:P, :], wb[lo:P, :], rr[lo:P, :])
                        wblk.append(wb)
                    ctx["wblk"] = wblk

                def s5():
                    wblk = ctx["wblk"]
                    we_ps = tp_pool.tile([P, 2 * C], FP, tag="tp")
                    a, bb = we_ps[:, 0:C], we_ps[:, C : 2 * C]
                    nc.tensor.matmul(a, wblk[0][:, 0:P], wprojT[0][:, :], start=True, stop=False)
                    nc.tensor.matmul(a, wblk[1][:, 0:P], wprojT[1][:, :], start=False, stop=True)
                    nc.tensor.matmul(bb, wblk[0][:, 64:C], wprojT[0][:, :], start=True, stop=False)
                    nc.tensor.matmul(bb, wblk[1][:, 64:C], wprojT[1][:, :], start=False, stop=True)
                    we_a = interm.tile([P, C], BF, tag="we_a")
                    we_b = interm.tile([P, C], BF, tag="we_b")
                    nc.vector.memset(we_b[0:64, :], 0.0)
                    nc.scalar.copy(we_a[:, :], a)
                    nc.scalar.copy(we_b[64:P, :], bb[64:P, :])
                    ctx["we_a"], ctx["we_b"] = we_a, we_b

                def s6():
                    we_a, we_b = ctx["we_a"], ctx["we_b"]
                    w2_ps = tp_pool.tile([P, 2 * C], FP, tag="tp")
                    a, bb = w2_ps[:, 0:C], w2_ps[:, C : 2 * C]
                    nc.tensor.matmul(a, wv[0][:, 0:P], we_a[:, :], start=True, stop=False)
                    nc.tensor.matmul(a, wv[1][:, 0:P], we_b[:, :], start=False, stop=True)
                    nc.tensor.matmul(bb, wv[0][:, 64:C], we_a[:, :], start=True, stop=False)
                    nc.tensor.matmul(bb, wv[1][:, 64:C], we_b[:, :], start=False, stop=True)
                    w2t_a = w2t_pool.tile([P, C], BF, tag="w2t_a")
                    w2t_b = w2t_pool.tile([P, C], BF, tag="w2t_b")
                    nc.scalar.copy(w2t_a[:, :], a)
                    nc.vector.tensor_copy(w2t_b[64:P, :], bb[64:P, :])
                    nc.vector.memset(w2t_b[0:64, :], 0.0)
                    st["w2t_a"], st["w2t_b"] = w2t_a, w2t_b

                return [s1, s2, s3, s4, s5, s6]

            def p2_group(b, g, free_g=False):
                st = state[b]
                xta, xtb = st["xta"], st["xtb"]
                w2t_a, w2t_b = st["w2t_a"], st["w2t_b"]
                ysb = ysb_pool.tile([P, TD, C], BF, tag="ysb")
                last = b == 1 and g == NG - 1
                for j2 in range(TD // 2):
                    # tail phase (b==1): rotate PSUM tiles through both pools so
                    # 6 buffers are in flight and the PE never waits on reuse.
                    pool = y_pool if (b == 0 or j2 % 2 == 0) else tp_pool
                    y_ps = pool.tile([P, 2, C], FP, tag="y" if pool is y_pool else "tp")
                    for k in range(2):
                        t = g * TD + j2 * 2 + k
                        nc.tensor.matmul(y_ps[:, k, :], xta[:, t, :], w2t_a[:, :], start=True, stop=False)
                        nc.tensor.matmul(y_ps[:, k, :], xtb[:, t, :], w2t_b[:, :], start=False, stop=True)
                    j0 = j2 * 2
                    if j2 % 2 == 0:
                        nc.scalar.copy(ysb[:, j0 : j0 + 2, :], y_ps[:, :, :])
                    else:
                        nc.vector.tensor_copy(ysb[:, j0 : j0 + 2, :], y_ps[:, :, :])
                    if last:
                        # drain the final group in [P,2,C] slices right behind
                        # each copy so the tail stub is one slice, not a group
                        eng = nc.sync if j2 % 2 == 0 else nc.scalar
                        eng.dma_start(
                            out=out_ap[b, :, g * TD + j0 : g * TD + j0 + 2, :],
                            in_=ysb[:, j0 : j0 + 2, :],
                        )
                if not last:
                    dst = out_ap[b, :, g * TD : (g + 1) * TD, :]
                    nc.sync.dma_start(out=dst, in_=ysb[:, :, :])

            # ---------------- emission schedule (keeps PE gap-free) --------
            # (p1_start(0) and the first two loads were issued before the
            # weight loads above)
            for g in range(NG):
                p1_group(0, g)
            # interlude(b0) stages interleaved with the first pass-1 groups of b1
            p1_start(1)
            st0 = interlude_stages(0)
            # cover plan: PE work emitted before each stage so its cross-engine
            # chain hides; s3 has no chain (PSUM to PSUM via SBUF weights), s4's
            # softmax chain is the longest and gets double cover.
            COVER = [1, 1, 0, 2, 1, 1]
            gi = 0
            for s, cov in zip(st0, COVER):
                for _ in range(cov):
                    p1_group(1, gi)
                    gi += 1
                s()
            # pass2(b0) interleaved with the rest of pass1(b1)
            g2i = 0
            for g in range(gi, NG):
                p1_group(1, g)
                p2_group(0, g2i)
                g2i += 1
            # interlude(b1) interleaved with the remaining pass2(b0) groups
            st1 = interlude_stages(1)
            for s, cov in zip(st1, COVER):
                for _ in range(cov):
                    if g2i < NG:
                        p2_group(0, g2i)
                        g2i += 1
                s()
            while g2i < NG:
                p2_group(0, g2i)
                g2i += 1
            for g in range(NG):
                p2_group(1, g)

    nc.compile()
    return nc


def _host_inputs(x, w_qkv, w_proj):
    w_q = w_qkv[0:C]
    w_k = w_qkv[C : 2 * C]
    w_v = w_qkv[2 * C : 3 * C]
    p = np.arange(C)
    mask = np.where((p[:, None] // M) == (p[None, :] // M), 0.0, MASK_NEG).astype(
        np.float32
    )
    common = {
        "wqT": np.ascontiguousarray(w_q.T) * TEMP,
        "wkT": np.ascontiguousarray(w_k.T),
        "wv": np.ascontiguousarray(w_v),
        "wprojT": np.ascontiguousarray(w_proj.T),
        "mask": mask,
        "ident": np.eye(P, dtype=np.float32),
    }
    # partition-major bf16 tiling: [B, NT, P, C] -> [B, P, NT, C]
    xt = (
        x.reshape(B, NT, P, C)
        .astype(np.float16)
        .transpose(0, 2, 1, 3)
    )
    in_maps = []
    for i in range(NCORES):
        m = dict(common)
        m["x"] = np.ascontiguousarray(xt[i * B_LOC : (i + 1) * B_LOC])
        in_maps.append(m)
    return in_maps


_CACHED_NC = None


def _get_nc():
    global _CACHED_NC
    if _CACHED_NC is None:
        _CACHED_NC = build_kernel()
    return _CACHED_NC


def kernel(x, w_qkv, w_proj, _trace=False, _results_out=None):
    x = np.ascontiguousarray(np.asarray(x, dtype=np.float32))
    w_qkv = np.asarray(w_qkv, dtype=np.float32)
    w_proj = np.asarray(w_proj, dtype=np.float32)
    nc = _get_nc()
    in_maps = _host_inputs(x, w_qkv, w_proj)
    res = run_bass_kernel_spmd(nc, in_maps, core_ids=list(range(NCORES)), trace=_trace)
    if _results_out is not None:
        _results_out.append(res)
    # out: [B_LOC, P, NT, C] partition-major -> [B_LOC, NT, P, C] -> [B, H, W, C]
    outs = [
        np.asarray(res.results[i]["out"]).astype(np.float32).transpose(0, 2, 1, 3)
        for i in range(NCORES)
    ]
    y = np.concatenate(outs, axis=0).reshape(B, H, W, C)
    return y.astype(np.float32)
